# revision 40
# baseline (speedup 1.0000x reference)
"""Bass/Trainium2 kernel for nn_Attention (Bahdanau-style attention).

  w1e   = enc @ W1.T                      [B, N, H]
  w2h   = h0 @ W2.T + b2                  [B, H]
  u     = tanh(w1e + w2h[:, None, :])     [B, N, H]
  logits= u @ V                           [B, N, 1]
  att   = softmax(logits, axis=1)
  out   = att^T @ enc                     [B, IN1]

Sharding: pure data-parallel over batch B=128 across 8 cores (16 batches
each); W1/W2/V replicated. No collectives.

Per-core dataflow (layout: tokens on partitions, H on free dim):
  - main matmul: stationary = enc^T tile [128 IN1, 128 tok] (host
    pre-transposed, bf16 shipped as uint16), moving = W1^T [128 IN1,
    512 H]; K=IN1=256 -> 2 accumulating matmuls per token tile; a 3rd
    K=1 matmul with an all-ones stationary row folds c = W2 h0 + b2
    (computed on device, broadcast to 128 partitions via a DRAM bounce)
    into the same PSUM accumulation. The 4 c-fold matmuls of a 4-tile
    group are row-packed at tile_position rows {0,32,64,96} so they run
    concurrently on the PE array.
  - tanh on ScalarE (PSUM->SBUF, bf16 out), 2 token-tiles per instr.
  - V-dot: one pair-wide tensor_tensor mult (u * V via a stride-0
    middle-dim broadcast AP), mostly on GpSimd, then a free-dim
    add-reduce -> logits columns [128, 16]. Pure-DVE pairs use one 3D
    tensor_reduce ([128,2,512] -> [128,2]); ~24/256 reduces go to
    ScalarE Identity+accum_out so ScalarE and VectorE stay balanced.
    (tensor_tensor_reduce and tensor_scalar+accum_out are broken on
    this toolchain: device wedge / BIR verifier reject.)
  - exp on ScalarE -> e [128, 16] bf16 (no max-subtract: |logits| <=
    ||V||_1 ~= 18, exp fits fp32/bf16 fine).
  - final weighted sum on PE: stationary = e column [128, 1], moving =
    enc natural tile [128 tok, 257] where column 256 is all-ones ->
    psum [1, 257] accumulates both att^T@enc AND the softmax denominator.
  - normalize by 1/S on ScalarE, DMA out per batch row.
"""

import os
import sys

for _p in ("/opt/trn_rl_repo",):
    if _p not in sys.path and os.path.isdir(_p):
        sys.path.insert(0, _p)

from contextlib import ExitStack

import ml_dtypes
import numpy as np

import concourse.bass as bass
from concourse import bacc, mybir, tile

B, N, IN1, IN2, H = 128, 2048, 256, 512, 512
NCORES = 8
BC = B // NCORES            # 16 batches per core
TOK = BC * N                # 32768 tokens per core
TPB = N // 128              # 16 token tiles per batch
NPAIR = TPB // 2            # 8 tile-pairs per batch
ENC_NW = 272                # padded natural width (257 used, 32B-aligned rows)

F32 = mybir.dt.float32
BF16 = mybir.dt.bfloat16

LAST_RUNNER = None

_CACHED_NC = None


class Runner:
    """Compile-once SPMD runner (replicates run_bass_via_pjrt's multi-core
    path) that keeps the jitted callable + device-resident inputs so
    repeated executions can be wall-clocked without compile/transfer."""

    def __init__(self, nc, in_maps):
        import jax
        from jax.experimental.shard_map import shard_map
        from jax.sharding import Mesh, NamedSharding, PartitionSpec

        from concourse import bass2jax, mybir as _mybir

        bass2jax.install_neuronx_cc_hook()
        self.jax = jax

        if not nc.is_finalized():
            nc.finalize()

        partition_name = (nc.partition_id_tensor.name
                          if nc.partition_id_tensor else None)
        in_names, out_names, out_avals, zero_outs = [], [], [], []
        for alloc in nc.m.functions[0].allocations:
            if not isinstance(alloc, _mybir.MemoryLocationSet):
                continue
            name = alloc.memorylocations[0].name
            if alloc.kind == "ExternalInput":
                if name != partition_name:
                    in_names.append(name)
            elif alloc.kind == "ExternalOutput":
                shape = tuple(alloc.tensor_shape)
                dtype = _mybir.dt.np(alloc.dtype)
                out_names.append(name)
                out_avals.append(jax.core.ShapedArray(shape, dtype))
                zero_outs.append(np.zeros(shape, dtype))
        n_params = len(in_names)
        all_in_names = list(in_names) + list(out_names)
        if partition_name is not None:
            all_in_names.append(partition_name)
        self.out_names = out_names

        def _body(*args):
            operands = list(args)
            if partition_name is not None:
                operands.append(bass2jax.partition_id_tensor())
            outs = bass2jax._bass_exec_p.bind(
                *operands,
                out_avals=tuple(out_avals),
                in_names=tuple(all_in_names),
                out_names=tuple(out_names),
                lowering_input_output_aliases=(),
                sim_require_finite=True,
                sim_require_nnan=True,
                nc=nc,
            )
            return tuple(outs)

        n_cores = len(in_maps)
        devices = jax.devices()[:n_cores]
        mesh = Mesh(np.asarray(devices), ("core",))
        spec = PartitionSpec("core")
        self.n_cores = n_cores
        self.out_avals = out_avals
        self.sharded = jax.jit(
            shard_map(_body, mesh=mesh,
                      in_specs=(spec,) * (n_params + len(out_names)),
                      out_specs=(spec,) * len(out_names),
                      check_rep=False),
            keep_unused=True,
        )

        def _body_chain(k):
            # k sequential executions chained through the output buffers:
            # each call's outputs become the next call's pre-zeroed output
            # operands, forcing true sequential execution in one dispatch.
            def f(*args):
                ins, zouts = args[:n_params], list(args[n_params:])
                for _ in range(k):
                    zouts = list(_body(*ins, *zouts))
                return tuple(zouts)
            return f

        self._chain_cache = {}
        self._mesh, self._spec = mesh, spec
        self._n_params = n_params
        self._shard_map, self._jit = shard_map, jax.jit
        self._body_chain = _body_chain
        sharding = NamedSharding(mesh, spec)
        self.dev_in = [
            jax.device_put(
                np.concatenate([np.asarray(in_maps[c][nm])
                                for c in range(n_cores)], axis=0), sharding)
            for nm in in_names
        ]
        self.dev_zeros = [
            jax.device_put(
                np.zeros((n_cores * z.shape[0], *z.shape[1:]), z.dtype), sharding)
            for z in zero_outs
        ]

    def run(self):
        out = self.sharded(*self.dev_in, *self.dev_zeros)
        self.jax.block_until_ready(out)
        return out

    def run_chain(self, k):
        # k async dispatches of the same executable; PJRT serializes them
        # on the device stream, so wall(k) - wall(1) ~= (k-1) * exec_time
        # (neuronx_cc_hook rejects >1 bass_exec per jitted module, so a
        # true in-graph chain is not compilable).
        out = None
        for _ in range(k):
            out = self.sharded(*self.dev_in, *self.dev_zeros)
        self.jax.block_until_ready(out)
        return out

    def outputs(self, out_arrs):
        return [
            {nm: np.asarray(out_arrs[i]).reshape(
                self.n_cores, *self.out_avals[i].shape)[c]
             for i, nm in enumerate(self.out_names)}
            for c in range(self.n_cores)
        ]


def build_nc(bc=BC, tpb=TPB):
    tok = bc * tpb * 128
    npair = tpb // 2
    nc = bacc.Bacc(None, target_bir_lowering=False)

    # NOTE: native bfloat16 ExternalInputs are mangled by the axon/PJRT
    # transfer path (measured: garbage values, device wedge). Ship bf16
    # bits as uint16 and bitcast on-chip.
    U16 = mybir.dt.uint16
    encT = nc.dram_tensor("encT", [IN1, tok], U16, kind="ExternalInput")
    encN = nc.dram_tensor("encN", [tok, ENC_NW], U16, kind="ExternalInput")
    w1t = nc.dram_tensor("w1t", [IN1, H], U16, kind="ExternalInput")
    h0t = nc.dram_tensor("h0t", [IN2, bc], U16, kind="ExternalInput")
    w2ta = nc.dram_tensor("w2ta", [IN2 + 1, H], U16, kind="ExternalInput")
    vb = nc.dram_tensor("vb", [128, H], U16, kind="ExternalInput")
    out = nc.dram_tensor("out", [bc, IN1], F32, kind="ExternalOutput")

    Tanh = mybir.ActivationFunctionType.Tanh
    Exp = mybir.ActivationFunctionType.Exp
    Copy = mybir.ActivationFunctionType.Copy
    Alu = mybir.AluOpType

    with tile.TileContext(nc) as tc, ExitStack() as ctx:
        consts = ctx.enter_context(tc.tile_pool(name="consts", bufs=1))
        etp = ctx.enter_context(tc.tile_pool(name="etp", bufs=3))
        enp = ctx.enter_context(tc.tile_pool(name="enp", bufs=6))
        upool = ctx.enter_context(tc.tile_pool(name="upool", bufs=4))
        lpool = ctx.enter_context(tc.tile_pool(name="lpool", bufs=2))
        epool = ctx.enter_context(tc.tile_pool(name="epool", bufs=2))
        spool = ctx.enter_context(tc.tile_pool(name="spool", bufs=4))
        zpool = ctx.enter_context(tc.tile_pool(name="zpool", bufs=3, space="PSUM"))
        opool = ctx.enter_context(tc.tile_pool(name="opool", bufs=2, space="PSUM"))

        # ---------------- prologue: constants ----------------
        sb_w1t = consts.tile([128, 2, H], BF16)
        for k in range(2):
            nc.sync.dma_start(out=sb_w1t[:, k, :].bitcast(U16),
                              in_=w1t[k * 128:(k + 1) * 128, :])
        sb_vb = consts.tile([128, H], BF16)
        nc.sync.dma_start(out=sb_vb.bitcast(U16), in_=vb[:, :])
        sb_ones = consts.tile([1, 128], BF16)
        nc.vector.memset(sb_ones, 1.0)

        sb_h0t = consts.tile([128, 4, bc], BF16)
        for k in range(4):
            nc.sync.dma_start(out=sb_h0t[:, k, :].bitcast(U16),
                              in_=h0t[k * 128:(k + 1) * 128, :])
        sb_w2ta = consts.tile([128, 5, H], BF16)
        for k in range(4):
            nc.sync.dma_start(out=sb_w2ta[:, k, :].bitcast(U16),
                              in_=w2ta[k * 128:(k + 1) * 128, :])
        nc.sync.dma_start(out=sb_w2ta[0:1, 4, :].bitcast(U16),
                          in_=w2ta[IN2:IN2 + 1, :])

        # c = h0 @ W2.T + b2  -> [16, 512] in PSUM
        psum_c = zpool.tile([bc, H], F32, tag="z")
        for k in range(4):
            nc.tensor.matmul(psum_c, sb_h0t[:, k, :], sb_w2ta[:, k, :],
                             start=(k == 0), stop=False)
        nc.tensor.matmul(psum_c, sb_ones[0:1, 0:bc], sb_w2ta[0:1, 4, :],
                         start=False, stop=True)
        sb_c16 = consts.tile([bc, H], BF16)
        nc.vector.tensor_copy(sb_c16, psum_c)
        # c rows -> DRAM bounce -> broadcast to all 128 partitions, so the
        # K=1 c-fold matmuls can be row-packed at tile_position rows
        # {0,32,64,96} (4 concurrent on the PE array).
        dpool = ctx.enter_context(tc.tile_pool(name="dpool", bufs=1, space="DRAM"))
        c_dram = dpool.tile([bc, H], BF16)
        nc.gpsimd.dma_start(out=c_dram[:, :], in_=sb_c16[:, :])
        crep = consts.tile([128, bc * H], BF16)
        c_flat0 = bass.AP(tensor=c_dram.tensor, offset=c_dram.offset,
                          ap=[[0, 128], [1, H]])
        nc.gpsimd.dma_start(out=crep[:, 0:H], in_=c_flat0)
        c_flat1 = bass.AP(tensor=c_dram.tensor, offset=c_dram.offset + H,
                          ap=[[0, 128], [1, (bc - 1) * H]])
        nc.gpsimd.dma_start(out=crep[:, H:], in_=c_flat1)
        sb_onesq = consts.tile([128, 128], BF16)
        nc.vector.memset(sb_onesq, 1.0)

        # ---------------- main pipeline ----------------
        for b in range(bc):
            sb_logits = lpool.tile([128, tpb], F32, tag="logits")
            for g8 in range(tpb // 8):             # 8 token tiles per DMA group
                tok8 = (b * tpb + g8 * 8) * 128
                sb_et = etp.tile([128, 2, 1024], BF16, tag="et")
                for k in range(2):
                    nc.sync.dma_start(
                        out=sb_et[:, k, :].bitcast(U16),
                        in_=encT[k * 128:(k + 1) * 128, tok8:tok8 + 1024])
                for sub in range(2):                # 4-tile compute sub-groups
                    grp = g8 * 2 + sub
                    pz0 = zpool.tile([128, 1024], F32, tag="z")
                    pz1 = zpool.tile([128, 1024], F32, tag="z")
                    pz = [pz0, pz1]
                    for q in range(4):              # main matmuls, 4 tiles
                        zs = pz[q // 2][:, (q % 2) * 512:(q % 2 + 1) * 512]
                        qq = sub * 4 + q
                        for k in range(2):
                            nc.tensor.matmul(
                                zs, sb_et[:, k, qq * 128:(qq + 1) * 128],
                                sb_w1t[:, k, :], start=(k == 0), stop=False)
                    for q in range(4):              # row-packed c-fold matmuls
                        zs = pz[q // 2][:, (q % 2) * 512:(q % 2 + 1) * 512]
                        nc.tensor.matmul(zs, sb_onesq[32 * q:32 * q + 1, :],
                                         crep[32 * q:32 * q + 1,
                                              b * H:(b + 1) * H],
                                         start=False, stop=True,
                                         tile_position=(32 * q, 0))
                    for hz in range(2):
                        sb_u = upool.tile([128, 1024], BF16, tag="u")
                        nc.scalar.activation(sb_u, pz[hz], Tanh)
                        sb_prod = upool.tile([128, 2, 512], BF16, tag="prod")
                        t0 = grp * 4 + hz * 2
                        # one pair-wide V multiply (V_bcast repeated along free)
                        mul_eng = nc.gpsimd if (t0 // 2) % 4 != 3 else nc.vector
                        mul_eng.tensor_tensor(
                            out=sb_prod,
                            in0=sb_u.rearrange("p (j f) -> p j f", j=2),
                            in1=bass.AP(tensor=sb_vb.tensor,
                                        offset=sb_vb.offset,
                                        ap=[sb_vb.ap[0], [0, 2],
                                            sb_vb.ap[1]]),
                            op=Alu.mult)
                        # ScalarE takes tile 5 every batch and tile 11 on
                        # odd batches (~24/256 reduces) to balance ACT~DVE
                        act_tiles = {5} if b % 2 == 0 else {5, 11}
                        if (t0 in act_tiles) or (t0 + 1 in act_tiles):
                            # mixed pair: per-tile reduces (one on ScalarE)
                            for half in range(2):
                                t_idx = t0 + half
                                lg = sb_logits[:, t_idx:t_idx + 1]
                                if t_idx in act_tiles:
                                    junk = upool.tile([128, 512], BF16,
                                                      tag="junk")
                                    nc.scalar.activation(
                                        junk, sb_prod[:, half, :],
                                        mybir.ActivationFunctionType.Identity,
                                        accum_out=lg)
                                else:
                                    nc.vector.tensor_reduce(
                                        out=lg, in_=sb_prod[:, half, :],
                                        op=Alu.add, axis=mybir.AxisListType.X)
                        else:
                            # pure-DVE pair: one 3D reduce -> two logit cols
                            nc.vector.tensor_reduce(
                                out=sb_logits[:, t0:t0 + 2], in_=sb_prod,
                                op=Alu.add, axis=mybir.AxisListType.X)

            sb_e = epool.tile([128, tpb], BF16, tag="e")
            if b == bc - 1:
                # last batch: exp in two chunks so its final matmuls start
                # before the last logits are reduced (shortens the drain tail)
                half_t = tpb // 2
                nc.scalar.activation(sb_e[:, 0:half_t],
                                     sb_logits[:, 0:half_t], Exp)
                nc.scalar.activation(sb_e[:, half_t:],
                                     sb_logits[:, half_t:], Exp)
            else:
                nc.scalar.activation(sb_e, sb_logits, Exp)

            psum_o = opool.tile([1, 257], F32, tag="o")
            for sg in range(tpb // 4):
                s0 = b * tpb + sg * 4
                sb_en = enp.tile([128, 4, ENC_NW], BF16, tag="en")
                nc.sync.dma_start(
                    out=sb_en.bitcast(U16),
                    in_=encN[s0 * 128:(s0 + 4) * 128, :].rearrange(
                        "(j p) c -> p j c", p=128))
                for j in range(4):
                    s = sg * 4 + j
                    nc.tensor.matmul(psum_o, sb_e[:, s:s + 1],
                                     sb_en[:, j, 0:257],
                                     start=(s == 0), stop=(s == tpb - 1))
            rS = spool.tile([1, 1], F32, tag="rs")
            nc.vector.reciprocal(rS, psum_o[0:1, 256:257])
            sb_out = spool.tile([1, IN1], F32, tag="obuf")
            nc.vector.tensor_scalar_mul(sb_out, psum_o[0:1, 0:256], rS)
            nc.sync.dma_start(out=out[b:b + 1, :], in_=sb_out)

    return nc


def _to_bf16(x):
    """bf16 bits as uint16 (native bf16 inputs are mangled by the
    transfer path - see build_nc note)."""
    return np.ascontiguousarray(x.astype(ml_dtypes.bfloat16)).view(np.uint16)


def kernel(**inputs):
    global LAST_RUNNER, _CACHED_NC
    enc = np.asarray(inputs["enc_outputs"], dtype=np.float32)   # [B, N, IN1]
    h0 = np.asarray(inputs["h0"], dtype=np.float32)             # [B, IN2]
    W1 = np.asarray(inputs["W1"], dtype=np.float32)             # [H, IN1]
    W2 = np.asarray(inputs["W2"], dtype=np.float32)             # [H, IN2]
    b2 = np.asarray(inputs["b2"], dtype=np.float32)             # [H]
    V = np.asarray(inputs["V"], dtype=np.float32)               # [H, 1]

    w1t = _to_bf16(W1.T)                                        # [IN1, H]
    w2ta = _to_bf16(np.concatenate([W2.T, b2[None, :]], 0))     # [IN2+1, H]
    vb = _to_bf16(np.broadcast_to(V.reshape(1, H), (128, H)))   # [128, H]

    in_maps = []
    for c in range(NCORES):
        enc_c = enc[c * BC:(c + 1) * BC]                        # [16, N, IN1]
        flat = enc_c.reshape(TOK, IN1)
        encT = _to_bf16(np.ascontiguousarray(flat.T))           # [IN1, TOK]
        encN = np.zeros((TOK, ENC_NW), dtype=ml_dtypes.bfloat16)
        encN[:, :IN1] = flat.astype(ml_dtypes.bfloat16)
        encN[:, IN1] = 1.0
        encN = encN.view(np.uint16)
        h0t = _to_bf16(h0[c * BC:(c + 1) * BC].T)               # [IN2, 16]
        in_maps.append({
            "encT": encT, "encN": encN, "w1t": w1t,
            "h0t": h0t, "w2ta": w2ta, "vb": vb,
        })

    if _CACHED_NC is None:
        _CACHED_NC = build_nc()
    nc = _CACHED_NC

    runner = Runner(nc, in_maps)
    LAST_RUNNER = runner
    results = runner.outputs(runner.run())
    out = np.concatenate([results[i]["out"] for i in range(NCORES)], axis=0)
    return out.astype(np.float32)


# revision 41
# speedup vs baseline: 1.0020x; 1.0020x over previous
"""Bass/Trainium2 kernel for nn_Attention (Bahdanau-style attention).

  w1e   = enc @ W1.T                      [B, N, H]
  w2h   = h0 @ W2.T + b2                  [B, H]
  u     = tanh(w1e + w2h[:, None, :])     [B, N, H]
  logits= u @ V                           [B, N, 1]
  att   = softmax(logits, axis=1)
  out   = att^T @ enc                     [B, IN1]

Sharding: pure data-parallel over batch B=128 across 8 cores (16 batches
each); W1/W2/V replicated. No collectives.

Per-core dataflow (layout: tokens on partitions, H on free dim):
  - main matmul: stationary = enc^T tile [128 IN1, 128 tok] (host
    pre-transposed, bf16 shipped as uint16), moving = W1^T [128 IN1,
    512 H]; K=IN1=256 -> 2 accumulating matmuls per token tile; a 3rd
    K=1 matmul with an all-ones stationary row folds c = W2 h0 + b2
    (computed on device, broadcast to 128 partitions via a DRAM bounce)
    into the same PSUM accumulation. The 4 c-fold matmuls of a 4-tile
    group are row-packed at tile_position rows {0,32,64,96} so they run
    concurrently on the PE array.
  - tanh on ScalarE (PSUM->SBUF, bf16 out), 2 token-tiles per instr.
  - V-dot: one pair-wide tensor_tensor mult (u * V via a stride-0
    middle-dim broadcast AP), mostly on GpSimd, then a free-dim
    add-reduce -> logits columns [128, 16]. Pure-DVE pairs use one 3D
    tensor_reduce ([128,2,512] -> [128,2]); ~24/256 reduces go to
    ScalarE Identity+accum_out so ScalarE and VectorE stay balanced.
    (tensor_tensor_reduce and tensor_scalar+accum_out are broken on
    this toolchain: device wedge / BIR verifier reject.)
  - exp on ScalarE -> e [128, 16] bf16 (no max-subtract: |logits| <=
    ||V||_1 ~= 18, exp fits fp32/bf16 fine).
  - final weighted sum on PE: stationary = e column [128, 1], moving =
    enc natural tile [128 tok, 257] where column 256 is all-ones ->
    psum [1, 257] accumulates both att^T@enc AND the softmax denominator.
  - normalize by 1/S on ScalarE, DMA out per batch row.
"""

import os
import sys

for _p in ("/opt/trn_rl_repo",):
    if _p not in sys.path and os.path.isdir(_p):
        sys.path.insert(0, _p)

from contextlib import ExitStack

import ml_dtypes
import numpy as np

import concourse.bass as bass
from concourse import bacc, mybir, tile

B, N, IN1, IN2, H = 128, 2048, 256, 512, 512
NCORES = 8
BC = B // NCORES            # 16 batches per core
TOK = BC * N                # 32768 tokens per core
TPB = N // 128              # 16 token tiles per batch
NPAIR = TPB // 2            # 8 tile-pairs per batch
ENC_NW = 272                # padded natural width (257 used, 32B-aligned rows)

F32 = mybir.dt.float32
BF16 = mybir.dt.bfloat16

LAST_RUNNER = None

_CACHED_NC = None


class Runner:
    """Compile-once SPMD runner (replicates run_bass_via_pjrt's multi-core
    path) that keeps the jitted callable + device-resident inputs so
    repeated executions can be wall-clocked without compile/transfer."""

    def __init__(self, nc, in_maps):
        import jax
        from jax.experimental.shard_map import shard_map
        from jax.sharding import Mesh, NamedSharding, PartitionSpec

        from concourse import bass2jax, mybir as _mybir

        bass2jax.install_neuronx_cc_hook()
        self.jax = jax

        if not nc.is_finalized():
            nc.finalize()

        partition_name = (nc.partition_id_tensor.name
                          if nc.partition_id_tensor else None)
        in_names, out_names, out_avals, zero_outs = [], [], [], []
        for alloc in nc.m.functions[0].allocations:
            if not isinstance(alloc, _mybir.MemoryLocationSet):
                continue
            name = alloc.memorylocations[0].name
            if alloc.kind == "ExternalInput":
                if name != partition_name:
                    in_names.append(name)
            elif alloc.kind == "ExternalOutput":
                shape = tuple(alloc.tensor_shape)
                dtype = _mybir.dt.np(alloc.dtype)
                out_names.append(name)
                out_avals.append(jax.core.ShapedArray(shape, dtype))
                zero_outs.append(np.zeros(shape, dtype))
        n_params = len(in_names)
        all_in_names = list(in_names) + list(out_names)
        if partition_name is not None:
            all_in_names.append(partition_name)
        self.out_names = out_names

        def _body(*args):
            operands = list(args)
            if partition_name is not None:
                operands.append(bass2jax.partition_id_tensor())
            outs = bass2jax._bass_exec_p.bind(
                *operands,
                out_avals=tuple(out_avals),
                in_names=tuple(all_in_names),
                out_names=tuple(out_names),
                lowering_input_output_aliases=(),
                sim_require_finite=True,
                sim_require_nnan=True,
                nc=nc,
            )
            return tuple(outs)

        n_cores = len(in_maps)
        devices = jax.devices()[:n_cores]
        mesh = Mesh(np.asarray(devices), ("core",))
        spec = PartitionSpec("core")
        self.n_cores = n_cores
        self.out_avals = out_avals
        self.sharded = jax.jit(
            shard_map(_body, mesh=mesh,
                      in_specs=(spec,) * (n_params + len(out_names)),
                      out_specs=(spec,) * len(out_names),
                      check_rep=False),
            keep_unused=True,
        )

        def _body_chain(k):
            # k sequential executions chained through the output buffers:
            # each call's outputs become the next call's pre-zeroed output
            # operands, forcing true sequential execution in one dispatch.
            def f(*args):
                ins, zouts = args[:n_params], list(args[n_params:])
                for _ in range(k):
                    zouts = list(_body(*ins, *zouts))
                return tuple(zouts)
            return f

        self._chain_cache = {}
        self._mesh, self._spec = mesh, spec
        self._n_params = n_params
        self._shard_map, self._jit = shard_map, jax.jit
        self._body_chain = _body_chain
        sharding = NamedSharding(mesh, spec)
        self.dev_in = [
            jax.device_put(
                np.concatenate([np.asarray(in_maps[c][nm])
                                for c in range(n_cores)], axis=0), sharding)
            for nm in in_names
        ]
        self.dev_zeros = [
            jax.device_put(
                np.zeros((n_cores * z.shape[0], *z.shape[1:]), z.dtype), sharding)
            for z in zero_outs
        ]

    def run(self):
        out = self.sharded(*self.dev_in, *self.dev_zeros)
        self.jax.block_until_ready(out)
        return out

    def run_chain(self, k):
        # k async dispatches of the same executable; PJRT serializes them
        # on the device stream, so wall(k) - wall(1) ~= (k-1) * exec_time
        # (neuronx_cc_hook rejects >1 bass_exec per jitted module, so a
        # true in-graph chain is not compilable).
        out = None
        for _ in range(k):
            out = self.sharded(*self.dev_in, *self.dev_zeros)
        self.jax.block_until_ready(out)
        return out

    def outputs(self, out_arrs):
        return [
            {nm: np.asarray(out_arrs[i]).reshape(
                self.n_cores, *self.out_avals[i].shape)[c]
             for i, nm in enumerate(self.out_names)}
            for c in range(self.n_cores)
        ]


def build_nc(bc=BC, tpb=TPB):
    tok = bc * tpb * 128
    npair = tpb // 2
    nc = bacc.Bacc(None, target_bir_lowering=False)

    # NOTE: native bfloat16 ExternalInputs are mangled by the axon/PJRT
    # transfer path (measured: garbage values, device wedge). Ship bf16
    # bits as uint16 and bitcast on-chip.
    U16 = mybir.dt.uint16
    encT = nc.dram_tensor("encT", [IN1, tok], U16, kind="ExternalInput")
    encN = nc.dram_tensor("encN", [tok, ENC_NW], U16, kind="ExternalInput")
    w1t = nc.dram_tensor("w1t", [IN1, H], U16, kind="ExternalInput")
    h0t = nc.dram_tensor("h0t", [IN2, bc], U16, kind="ExternalInput")
    w2ta = nc.dram_tensor("w2ta", [IN2 + 1, H], U16, kind="ExternalInput")
    vb = nc.dram_tensor("vb", [128, H], U16, kind="ExternalInput")
    out = nc.dram_tensor("out", [bc, IN1], F32, kind="ExternalOutput")

    Tanh = mybir.ActivationFunctionType.Tanh
    Exp = mybir.ActivationFunctionType.Exp
    Copy = mybir.ActivationFunctionType.Copy
    Alu = mybir.AluOpType

    with tile.TileContext(nc) as tc, ExitStack() as ctx:
        consts = ctx.enter_context(tc.tile_pool(name="consts", bufs=1))
        etp = ctx.enter_context(tc.tile_pool(name="etp", bufs=3))
        enp = ctx.enter_context(tc.tile_pool(name="enp", bufs=6))
        upool = ctx.enter_context(tc.tile_pool(name="upool", bufs=4))
        lpool = ctx.enter_context(tc.tile_pool(name="lpool", bufs=2))
        epool = ctx.enter_context(tc.tile_pool(name="epool", bufs=2))
        spool = ctx.enter_context(tc.tile_pool(name="spool", bufs=4))
        zpool = ctx.enter_context(tc.tile_pool(name="zpool", bufs=3, space="PSUM"))
        opool = ctx.enter_context(tc.tile_pool(name="opool", bufs=2, space="PSUM"))

        # ---------------- prologue: constants ----------------
        sb_w1t = consts.tile([128, 2, H], BF16)
        for k in range(2):
            nc.sync.dma_start(out=sb_w1t[:, k, :].bitcast(U16),
                              in_=w1t[k * 128:(k + 1) * 128, :])
        sb_vb = consts.tile([128, H], BF16)
        nc.sync.dma_start(out=sb_vb.bitcast(U16), in_=vb[:, :])
        sb_ones = consts.tile([1, 128], BF16)
        nc.vector.memset(sb_ones, 1.0)

        sb_h0t = consts.tile([128, 4, bc], BF16)
        for k in range(4):
            nc.sync.dma_start(out=sb_h0t[:, k, :].bitcast(U16),
                              in_=h0t[k * 128:(k + 1) * 128, :])
        sb_w2ta = consts.tile([128, 5, H], BF16)
        for k in range(4):
            nc.sync.dma_start(out=sb_w2ta[:, k, :].bitcast(U16),
                              in_=w2ta[k * 128:(k + 1) * 128, :])
        nc.sync.dma_start(out=sb_w2ta[0:1, 4, :].bitcast(U16),
                          in_=w2ta[IN2:IN2 + 1, :])

        # c = h0 @ W2.T + b2  -> [16, 512] in PSUM
        psum_c = zpool.tile([bc, H], F32, tag="z")
        for k in range(4):
            nc.tensor.matmul(psum_c, sb_h0t[:, k, :], sb_w2ta[:, k, :],
                             start=(k == 0), stop=False)
        nc.tensor.matmul(psum_c, sb_ones[0:1, 0:bc], sb_w2ta[0:1, 4, :],
                         start=False, stop=True)
        sb_c16 = consts.tile([bc, H], BF16)
        nc.vector.tensor_copy(sb_c16, psum_c)
        # c rows -> DRAM bounce -> broadcast to all 128 partitions, so the
        # K=1 c-fold matmuls can be row-packed at tile_position rows
        # {0,32,64,96} (4 concurrent on the PE array).
        dpool = ctx.enter_context(tc.tile_pool(name="dpool", bufs=1, space="DRAM"))
        c_dram = dpool.tile([bc, H], BF16)
        nc.gpsimd.dma_start(out=c_dram[:, :], in_=sb_c16[:, :])
        crep = consts.tile([128, bc * H], BF16)
        c_flat0 = bass.AP(tensor=c_dram.tensor, offset=c_dram.offset,
                          ap=[[0, 128], [1, H]])
        nc.gpsimd.dma_start(out=crep[:, 0:H], in_=c_flat0)
        c_flat1 = bass.AP(tensor=c_dram.tensor, offset=c_dram.offset + H,
                          ap=[[0, 128], [1, (bc - 1) * H]])
        nc.gpsimd.dma_start(out=crep[:, H:], in_=c_flat1)
        sb_onesq = consts.tile([128, 128], BF16)
        nc.vector.memset(sb_onesq, 1.0)

        # ---------------- main pipeline ----------------
        for b in range(bc):
            sb_logits = lpool.tile([128, tpb], F32, tag="logits")
            for g8 in range(tpb // 8):             # 8 token tiles per DMA group
                tok8 = (b * tpb + g8 * 8) * 128
                sb_et = etp.tile([128, 2, 1024], BF16, tag="et")
                for k in range(2):
                    nc.sync.dma_start(
                        out=sb_et[:, k, :].bitcast(U16),
                        in_=encT[k * 128:(k + 1) * 128, tok8:tok8 + 1024])
                for sub in range(2):                # 4-tile compute sub-groups
                    grp = g8 * 2 + sub
                    pz0 = zpool.tile([128, 1024], F32, tag="z")
                    pz1 = zpool.tile([128, 1024], F32, tag="z")
                    pz = [pz0, pz1]
                    for q in range(4):              # main matmuls, 4 tiles
                        zs = pz[q // 2][:, (q % 2) * 512:(q % 2 + 1) * 512]
                        qq = sub * 4 + q
                        for k in range(2):
                            nc.tensor.matmul(
                                zs, sb_et[:, k, qq * 128:(qq + 1) * 128],
                                sb_w1t[:, k, :], start=(k == 0), stop=False)
                    first_grp = (b == 0 and g8 == 0 and sub == 0)
                    for q in range(4):              # row-packed c-fold matmuls
                        zs = pz[q // 2][:, (q % 2) * 512:(q % 2 + 1) * 512]
                        if first_grp:
                            # first group reads c directly from sb_c16 row 0
                            # (partition 0), skipping the crep DMA-chain
                            # latency at kernel startup
                            nc.tensor.matmul(zs, sb_onesq[0:1, :],
                                             sb_c16[0:1, :],
                                             start=False, stop=True)
                        else:
                            nc.tensor.matmul(zs, sb_onesq[32 * q:32 * q + 1, :],
                                             crep[32 * q:32 * q + 1,
                                                  b * H:(b + 1) * H],
                                             start=False, stop=True,
                                             tile_position=(32 * q, 0))
                    for hz in range(2):
                        sb_u = upool.tile([128, 1024], BF16, tag="u")
                        nc.scalar.activation(sb_u, pz[hz], Tanh)
                        sb_prod = upool.tile([128, 2, 512], BF16, tag="prod")
                        t0 = grp * 4 + hz * 2
                        # one pair-wide V multiply (V_bcast repeated along free)
                        mul_eng = nc.gpsimd if (t0 // 2) % 4 != 3 else nc.vector
                        mul_eng.tensor_tensor(
                            out=sb_prod,
                            in0=sb_u.rearrange("p (j f) -> p j f", j=2),
                            in1=bass.AP(tensor=sb_vb.tensor,
                                        offset=sb_vb.offset,
                                        ap=[sb_vb.ap[0], [0, 2],
                                            sb_vb.ap[1]]),
                            op=Alu.mult)
                        # ScalarE takes tile 5 every batch and tile 11 on
                        # odd batches (~24/256 reduces) to balance ACT~DVE
                        act_tiles = {5} if b % 2 == 0 else {5, 11}
                        if (t0 in act_tiles) or (t0 + 1 in act_tiles):
                            # mixed pair: per-tile reduces (one on ScalarE)
                            for half in range(2):
                                t_idx = t0 + half
                                lg = sb_logits[:, t_idx:t_idx + 1]
                                if t_idx in act_tiles:
                                    junk = upool.tile([128, 512], BF16,
                                                      tag="junk")
                                    nc.scalar.activation(
                                        junk, sb_prod[:, half, :],
                                        mybir.ActivationFunctionType.Identity,
                                        accum_out=lg)
                                else:
                                    nc.vector.tensor_reduce(
                                        out=lg, in_=sb_prod[:, half, :],
                                        op=Alu.add, axis=mybir.AxisListType.X)
                        else:
                            # pure-DVE pair: one 3D reduce -> two logit cols
                            nc.vector.tensor_reduce(
                                out=sb_logits[:, t0:t0 + 2], in_=sb_prod,
                                op=Alu.add, axis=mybir.AxisListType.X)

            sb_e = epool.tile([128, tpb], BF16, tag="e")
            if b == bc - 1:
                # last batch: exp in two chunks so its final matmuls start
                # before the last logits are reduced (shortens the drain tail)
                half_t = tpb // 2
                nc.scalar.activation(sb_e[:, 0:half_t],
                                     sb_logits[:, 0:half_t], Exp)
                nc.scalar.activation(sb_e[:, half_t:],
                                     sb_logits[:, half_t:], Exp)
            else:
                nc.scalar.activation(sb_e, sb_logits, Exp)

            psum_o = opool.tile([1, 257], F32, tag="o")
            for sg in range(tpb // 4):
                s0 = b * tpb + sg * 4
                sb_en = enp.tile([128, 4, ENC_NW], BF16, tag="en")
                nc.sync.dma_start(
                    out=sb_en.bitcast(U16),
                    in_=encN[s0 * 128:(s0 + 4) * 128, :].rearrange(
                        "(j p) c -> p j c", p=128))
                for j in range(4):
                    s = sg * 4 + j
                    nc.tensor.matmul(psum_o, sb_e[:, s:s + 1],
                                     sb_en[:, j, 0:257],
                                     start=(s == 0), stop=(s == tpb - 1))
            rS = spool.tile([1, 1], F32, tag="rs")
            nc.vector.reciprocal(rS, psum_o[0:1, 256:257])
            sb_out = spool.tile([1, IN1], F32, tag="obuf")
            nc.vector.tensor_scalar_mul(sb_out, psum_o[0:1, 0:256], rS)
            nc.sync.dma_start(out=out[b:b + 1, :], in_=sb_out)

    return nc


def _to_bf16(x):
    """bf16 bits as uint16 (native bf16 inputs are mangled by the
    transfer path - see build_nc note)."""
    return np.ascontiguousarray(x.astype(ml_dtypes.bfloat16)).view(np.uint16)


def kernel(**inputs):
    global LAST_RUNNER, _CACHED_NC
    enc = np.asarray(inputs["enc_outputs"], dtype=np.float32)   # [B, N, IN1]
    h0 = np.asarray(inputs["h0"], dtype=np.float32)             # [B, IN2]
    W1 = np.asarray(inputs["W1"], dtype=np.float32)             # [H, IN1]
    W2 = np.asarray(inputs["W2"], dtype=np.float32)             # [H, IN2]
    b2 = np.asarray(inputs["b2"], dtype=np.float32)             # [H]
    V = np.asarray(inputs["V"], dtype=np.float32)               # [H, 1]

    w1t = _to_bf16(W1.T)                                        # [IN1, H]
    w2ta = _to_bf16(np.concatenate([W2.T, b2[None, :]], 0))     # [IN2+1, H]
    vb = _to_bf16(np.broadcast_to(V.reshape(1, H), (128, H)))   # [128, H]

    in_maps = []
    for c in range(NCORES):
        enc_c = enc[c * BC:(c + 1) * BC]                        # [16, N, IN1]
        flat = enc_c.reshape(TOK, IN1)
        encT = _to_bf16(np.ascontiguousarray(flat.T))           # [IN1, TOK]
        encN = np.zeros((TOK, ENC_NW), dtype=ml_dtypes.bfloat16)
        encN[:, :IN1] = flat.astype(ml_dtypes.bfloat16)
        encN[:, IN1] = 1.0
        encN = encN.view(np.uint16)
        h0t = _to_bf16(h0[c * BC:(c + 1) * BC].T)               # [IN2, 16]
        in_maps.append({
            "encT": encT, "encN": encN, "w1t": w1t,
            "h0t": h0t, "w2ta": w2ta, "vb": vb,
        })

    if _CACHED_NC is None:
        _CACHED_NC = build_nc()
    nc = _CACHED_NC

    runner = Runner(nc, in_maps)
    LAST_RUNNER = runner
    results = runner.outputs(runner.run())
    out = np.concatenate([results[i]["out"] for i in range(NCORES)], axis=0)
    return out.astype(np.float32)


# revision 42
# speedup vs baseline: 1.0064x; 1.0044x over previous
"""Bass/Trainium2 kernel for nn_Attention (Bahdanau-style attention).

  w1e   = enc @ W1.T                      [B, N, H]
  w2h   = h0 @ W2.T + b2                  [B, H]
  u     = tanh(w1e + w2h[:, None, :])     [B, N, H]
  logits= u @ V                           [B, N, 1]
  att   = softmax(logits, axis=1)
  out   = att^T @ enc                     [B, IN1]

Sharding: pure data-parallel over batch B=128 across 8 cores (16 batches
each); W1/W2/V replicated. No collectives.

Per-core dataflow (layout: tokens on partitions, H on free dim):
  - main matmul: stationary = enc^T tile [128 IN1, 128 tok] (host
    pre-transposed, bf16 shipped as uint16), moving = W1^T [128 IN1,
    512 H]; K=IN1=256 -> 2 accumulating matmuls per token tile; a 3rd
    K=1 matmul with an all-ones stationary row folds c = W2 h0 + b2
    (computed on device, broadcast to 128 partitions via a DRAM bounce)
    into the same PSUM accumulation. The 4 c-fold matmuls of a 4-tile
    group are row-packed at tile_position rows {0,32,64,96} so they run
    concurrently on the PE array.
  - tanh on ScalarE (PSUM->SBUF, bf16 out), 2 token-tiles per instr.
  - V-dot: one pair-wide tensor_tensor mult (u * V via a stride-0
    middle-dim broadcast AP), mostly on GpSimd, then a free-dim
    add-reduce -> logits columns [128, 16]. Pure-DVE pairs use one 3D
    tensor_reduce ([128,2,512] -> [128,2]); ~24/256 reduces go to
    ScalarE Identity+accum_out so ScalarE and VectorE stay balanced.
    (tensor_tensor_reduce and tensor_scalar+accum_out are broken on
    this toolchain: device wedge / BIR verifier reject.)
  - exp on ScalarE -> e [128, 16] bf16 (no max-subtract: |logits| <=
    ||V||_1 ~= 18, exp fits fp32/bf16 fine).
  - final weighted sum on PE: stationary = e column [128, 1], moving =
    enc natural tile [128 tok, 257] where column 256 is all-ones ->
    psum [1, 257] accumulates both att^T@enc AND the softmax denominator.
  - normalize by 1/S on ScalarE, DMA out per batch row.
"""

import os
import sys

for _p in ("/opt/trn_rl_repo",):
    if _p not in sys.path and os.path.isdir(_p):
        sys.path.insert(0, _p)

from contextlib import ExitStack

import ml_dtypes
import numpy as np

import concourse.bass as bass
from concourse import bacc, mybir, tile

B, N, IN1, IN2, H = 128, 2048, 256, 512, 512
NCORES = 8
BC = B // NCORES            # 16 batches per core
TOK = BC * N                # 32768 tokens per core
TPB = N // 128              # 16 token tiles per batch
NPAIR = TPB // 2            # 8 tile-pairs per batch
ENC_NW = 272                # padded natural width (257 used, 32B-aligned rows)

F32 = mybir.dt.float32
BF16 = mybir.dt.bfloat16

LAST_RUNNER = None

_CACHED_NC = None


class Runner:
    """Compile-once SPMD runner (replicates run_bass_via_pjrt's multi-core
    path) that keeps the jitted callable + device-resident inputs so
    repeated executions can be wall-clocked without compile/transfer."""

    def __init__(self, nc, in_maps):
        import jax
        from jax.experimental.shard_map import shard_map
        from jax.sharding import Mesh, NamedSharding, PartitionSpec

        from concourse import bass2jax, mybir as _mybir

        bass2jax.install_neuronx_cc_hook()
        self.jax = jax

        if not nc.is_finalized():
            nc.finalize()

        partition_name = (nc.partition_id_tensor.name
                          if nc.partition_id_tensor else None)
        in_names, out_names, out_avals, zero_outs = [], [], [], []
        for alloc in nc.m.functions[0].allocations:
            if not isinstance(alloc, _mybir.MemoryLocationSet):
                continue
            name = alloc.memorylocations[0].name
            if alloc.kind == "ExternalInput":
                if name != partition_name:
                    in_names.append(name)
            elif alloc.kind == "ExternalOutput":
                shape = tuple(alloc.tensor_shape)
                dtype = _mybir.dt.np(alloc.dtype)
                out_names.append(name)
                out_avals.append(jax.core.ShapedArray(shape, dtype))
                zero_outs.append(np.zeros(shape, dtype))
        n_params = len(in_names)
        all_in_names = list(in_names) + list(out_names)
        if partition_name is not None:
            all_in_names.append(partition_name)
        self.out_names = out_names

        def _body(*args):
            operands = list(args)
            if partition_name is not None:
                operands.append(bass2jax.partition_id_tensor())
            outs = bass2jax._bass_exec_p.bind(
                *operands,
                out_avals=tuple(out_avals),
                in_names=tuple(all_in_names),
                out_names=tuple(out_names),
                lowering_input_output_aliases=(),
                sim_require_finite=True,
                sim_require_nnan=True,
                nc=nc,
            )
            return tuple(outs)

        n_cores = len(in_maps)
        devices = jax.devices()[:n_cores]
        mesh = Mesh(np.asarray(devices), ("core",))
        spec = PartitionSpec("core")
        self.n_cores = n_cores
        self.out_avals = out_avals
        self.sharded = jax.jit(
            shard_map(_body, mesh=mesh,
                      in_specs=(spec,) * (n_params + len(out_names)),
                      out_specs=(spec,) * len(out_names),
                      check_rep=False),
            keep_unused=True,
        )

        def _body_chain(k):
            # k sequential executions chained through the output buffers:
            # each call's outputs become the next call's pre-zeroed output
            # operands, forcing true sequential execution in one dispatch.
            def f(*args):
                ins, zouts = args[:n_params], list(args[n_params:])
                for _ in range(k):
                    zouts = list(_body(*ins, *zouts))
                return tuple(zouts)
            return f

        self._chain_cache = {}
        self._mesh, self._spec = mesh, spec
        self._n_params = n_params
        self._shard_map, self._jit = shard_map, jax.jit
        self._body_chain = _body_chain
        sharding = NamedSharding(mesh, spec)
        self.dev_in = [
            jax.device_put(
                np.concatenate([np.asarray(in_maps[c][nm])
                                for c in range(n_cores)], axis=0), sharding)
            for nm in in_names
        ]
        self.dev_zeros = [
            jax.device_put(
                np.zeros((n_cores * z.shape[0], *z.shape[1:]), z.dtype), sharding)
            for z in zero_outs
        ]

    def run(self):
        out = self.sharded(*self.dev_in, *self.dev_zeros)
        self.jax.block_until_ready(out)
        return out

    def run_chain(self, k):
        # k async dispatches of the same executable; PJRT serializes them
        # on the device stream, so wall(k) - wall(1) ~= (k-1) * exec_time
        # (neuronx_cc_hook rejects >1 bass_exec per jitted module, so a
        # true in-graph chain is not compilable).
        out = None
        for _ in range(k):
            out = self.sharded(*self.dev_in, *self.dev_zeros)
        self.jax.block_until_ready(out)
        return out

    def outputs(self, out_arrs):
        return [
            {nm: np.asarray(out_arrs[i]).reshape(
                self.n_cores, *self.out_avals[i].shape)[c]
             for i, nm in enumerate(self.out_names)}
            for c in range(self.n_cores)
        ]


def build_nc(bc=BC, tpb=TPB):
    tok = bc * tpb * 128
    npair = tpb // 2
    nc = bacc.Bacc(None, target_bir_lowering=False)

    # NOTE: native bfloat16 ExternalInputs are mangled by the axon/PJRT
    # transfer path (measured: garbage values, device wedge). Ship bf16
    # bits as uint16 and bitcast on-chip.
    U16 = mybir.dt.uint16
    encT = nc.dram_tensor("encT", [IN1, tok], U16, kind="ExternalInput")
    encN = nc.dram_tensor("encN", [tok, ENC_NW], U16, kind="ExternalInput")
    w1t = nc.dram_tensor("w1t", [IN1, H], U16, kind="ExternalInput")
    h0t = nc.dram_tensor("h0t", [IN2, bc], U16, kind="ExternalInput")
    w2ta = nc.dram_tensor("w2ta", [IN2 + 1, H], U16, kind="ExternalInput")
    vb = nc.dram_tensor("vb", [128, H], U16, kind="ExternalInput")
    out = nc.dram_tensor("out", [bc, IN1], F32, kind="ExternalOutput")

    Tanh = mybir.ActivationFunctionType.Tanh
    Exp = mybir.ActivationFunctionType.Exp
    Copy = mybir.ActivationFunctionType.Copy
    Alu = mybir.AluOpType

    with tile.TileContext(nc) as tc, ExitStack() as ctx:
        consts = ctx.enter_context(tc.tile_pool(name="consts", bufs=1))
        etp = ctx.enter_context(tc.tile_pool(name="etp", bufs=3))
        enp = ctx.enter_context(tc.tile_pool(name="enp", bufs=6))
        upool = ctx.enter_context(tc.tile_pool(name="upool", bufs=4))
        lpool = ctx.enter_context(tc.tile_pool(name="lpool", bufs=2))
        epool = ctx.enter_context(tc.tile_pool(name="epool", bufs=2))
        spool = ctx.enter_context(tc.tile_pool(name="spool", bufs=4))
        zpool = ctx.enter_context(tc.tile_pool(name="zpool", bufs=3, space="PSUM"))
        opool = ctx.enter_context(tc.tile_pool(name="opool", bufs=2, space="PSUM"))

        # ---------------- prologue: constants ----------------
        sb_w1t = consts.tile([128, 2, H], BF16)
        for k in range(2):
            nc.sync.dma_start(out=sb_w1t[:, k, :].bitcast(U16),
                              in_=w1t[k * 128:(k + 1) * 128, :])
        sb_vb = consts.tile([128, H], BF16)
        sb_ones = consts.tile([1, 128], BF16)
        nc.vector.memset(sb_ones, 1.0)

        sb_h0t = consts.tile([128, 4, bc], BF16)
        for k in range(4):
            nc.sync.dma_start(out=sb_h0t[:, k, :].bitcast(U16),
                              in_=h0t[k * 128:(k + 1) * 128, :])
        sb_w2ta = consts.tile([128, 5, H], BF16)
        for k in range(4):
            nc.sync.dma_start(out=sb_w2ta[:, k, :].bitcast(U16),
                              in_=w2ta[k * 128:(k + 1) * 128, :])
        nc.sync.dma_start(out=sb_w2ta[0:1, 4, :].bitcast(U16),
                          in_=w2ta[IN2:IN2 + 1, :])

        # c = h0 @ W2.T + b2  -> [16, 512] in PSUM
        psum_c = zpool.tile([bc, H], F32, tag="z")
        for k in range(4):
            nc.tensor.matmul(psum_c, sb_h0t[:, k, :], sb_w2ta[:, k, :],
                             start=(k == 0), stop=False)
        nc.tensor.matmul(psum_c, sb_ones[0:1, 0:bc], sb_w2ta[0:1, 4, :],
                         start=False, stop=True)
        sb_c16 = consts.tile([bc, H], BF16)
        nc.vector.tensor_copy(sb_c16, psum_c)
        # c rows -> DRAM bounce -> broadcast to all 128 partitions, so the
        # K=1 c-fold matmuls can be row-packed at tile_position rows
        # {0,32,64,96} (4 concurrent on the PE array).
        dpool = ctx.enter_context(tc.tile_pool(name="dpool", bufs=1, space="DRAM"))
        c_dram = dpool.tile([bc, H], BF16)
        nc.gpsimd.dma_start(out=c_dram[:, :], in_=sb_c16[:, :])
        crep = consts.tile([128, bc * H], BF16)
        c_flat0 = bass.AP(tensor=c_dram.tensor, offset=c_dram.offset,
                          ap=[[0, 128], [1, H]])
        nc.gpsimd.dma_start(out=crep[:, 0:H], in_=c_flat0)
        c_flat1 = bass.AP(tensor=c_dram.tensor, offset=c_dram.offset + H,
                          ap=[[0, 128], [1, (bc - 1) * H]])
        nc.gpsimd.dma_start(out=crep[:, H:], in_=c_flat1)
        sb_onesq = consts.tile([128, 128], BF16)
        nc.vector.memset(sb_onesq, 1.0)
        nc.sync.dma_start(out=sb_vb.bitcast(U16), in_=vb[:, :])

        # ---------------- main pipeline ----------------
        for b in range(bc):
            sb_logits = lpool.tile([128, tpb], F32, tag="logits")
            for g8 in range(tpb // 8):             # 8 token tiles per DMA group
                tok8 = (b * tpb + g8 * 8) * 128
                sb_et = etp.tile([128, 2, 1024], BF16, tag="et")
                for k in range(2):
                    nc.sync.dma_start(
                        out=sb_et[:, k, :].bitcast(U16),
                        in_=encT[k * 128:(k + 1) * 128, tok8:tok8 + 1024])
                for sub in range(2):                # 4-tile compute sub-groups
                    grp = g8 * 2 + sub
                    pz0 = zpool.tile([128, 1024], F32, tag="z")
                    pz1 = zpool.tile([128, 1024], F32, tag="z")
                    pz = [pz0, pz1]
                    for q in range(4):              # main matmuls, 4 tiles
                        zs = pz[q // 2][:, (q % 2) * 512:(q % 2 + 1) * 512]
                        qq = sub * 4 + q
                        for k in range(2):
                            nc.tensor.matmul(
                                zs, sb_et[:, k, qq * 128:(qq + 1) * 128],
                                sb_w1t[:, k, :], start=(k == 0), stop=False)
                    first_grp = (b == 0 and g8 == 0 and sub == 0)
                    for q in range(4):              # row-packed c-fold matmuls
                        zs = pz[q // 2][:, (q % 2) * 512:(q % 2 + 1) * 512]
                        if first_grp:
                            # first group reads c directly from sb_c16 row 0
                            # (partition 0), skipping the crep DMA-chain
                            # latency at kernel startup
                            nc.tensor.matmul(zs, sb_onesq[0:1, :],
                                             sb_c16[0:1, :],
                                             start=False, stop=True)
                        else:
                            nc.tensor.matmul(zs, sb_onesq[32 * q:32 * q + 1, :],
                                             crep[32 * q:32 * q + 1,
                                                  b * H:(b + 1) * H],
                                             start=False, stop=True,
                                             tile_position=(32 * q, 0))
                    for hz in range(2):
                        sb_u = upool.tile([128, 1024], BF16, tag="u")
                        nc.scalar.activation(sb_u, pz[hz], Tanh)
                        sb_prod = upool.tile([128, 2, 512], BF16, tag="prod")
                        t0 = grp * 4 + hz * 2
                        # one pair-wide V multiply (V_bcast repeated along free)
                        mul_eng = nc.gpsimd if (t0 // 2) % 4 != 3 else nc.vector
                        mul_eng.tensor_tensor(
                            out=sb_prod,
                            in0=sb_u.rearrange("p (j f) -> p j f", j=2),
                            in1=bass.AP(tensor=sb_vb.tensor,
                                        offset=sb_vb.offset,
                                        ap=[sb_vb.ap[0], [0, 2],
                                            sb_vb.ap[1]]),
                            op=Alu.mult)
                        # ScalarE takes tile 5 every batch and tile 11 on
                        # odd batches (~24/256 reduces) to balance ACT~DVE
                        act_tiles = {5} if b % 2 == 0 else {5, 11}
                        if (t0 in act_tiles) or (t0 + 1 in act_tiles):
                            # mixed pair: per-tile reduces (one on ScalarE)
                            for half in range(2):
                                t_idx = t0 + half
                                lg = sb_logits[:, t_idx:t_idx + 1]
                                if t_idx in act_tiles:
                                    junk = upool.tile([128, 512], BF16,
                                                      tag="junk")
                                    nc.scalar.activation(
                                        junk, sb_prod[:, half, :],
                                        mybir.ActivationFunctionType.Identity,
                                        accum_out=lg)
                                else:
                                    nc.vector.tensor_reduce(
                                        out=lg, in_=sb_prod[:, half, :],
                                        op=Alu.add, axis=mybir.AxisListType.X)
                        else:
                            # pure-DVE pair: one 3D reduce -> two logit cols
                            nc.vector.tensor_reduce(
                                out=sb_logits[:, t0:t0 + 2], in_=sb_prod,
                                op=Alu.add, axis=mybir.AxisListType.X)

            sb_e = epool.tile([128, tpb], BF16, tag="e")
            if b == bc - 1:
                # last batch: exp in two chunks so its final matmuls start
                # before the last logits are reduced (shortens the drain tail)
                half_t = tpb // 2
                nc.scalar.activation(sb_e[:, 0:half_t],
                                     sb_logits[:, 0:half_t], Exp)
                nc.scalar.activation(sb_e[:, half_t:],
                                     sb_logits[:, half_t:], Exp)
            else:
                nc.scalar.activation(sb_e, sb_logits, Exp)

            psum_o = opool.tile([1, 257], F32, tag="o")
            for sg in range(tpb // 4):
                s0 = b * tpb + sg * 4
                sb_en = enp.tile([128, 4, ENC_NW], BF16, tag="en")
                nc.sync.dma_start(
                    out=sb_en.bitcast(U16),
                    in_=encN[s0 * 128:(s0 + 4) * 128, :].rearrange(
                        "(j p) c -> p j c", p=128))
                for j in range(4):
                    s = sg * 4 + j
                    nc.tensor.matmul(psum_o, sb_e[:, s:s + 1],
                                     sb_en[:, j, 0:257],
                                     start=(s == 0), stop=(s == tpb - 1))
            rS = spool.tile([1, 1], F32, tag="rs")
            nc.vector.reciprocal(rS, psum_o[0:1, 256:257])
            sb_out = spool.tile([1, IN1], F32, tag="obuf")
            nc.vector.tensor_scalar_mul(sb_out, psum_o[0:1, 0:256], rS)
            nc.sync.dma_start(out=out[b:b + 1, :], in_=sb_out)

    return nc


def _to_bf16(x):
    """bf16 bits as uint16 (native bf16 inputs are mangled by the
    transfer path - see build_nc note)."""
    return np.ascontiguousarray(x.astype(ml_dtypes.bfloat16)).view(np.uint16)


def kernel(**inputs):
    global LAST_RUNNER, _CACHED_NC
    enc = np.asarray(inputs["enc_outputs"], dtype=np.float32)   # [B, N, IN1]
    h0 = np.asarray(inputs["h0"], dtype=np.float32)             # [B, IN2]
    W1 = np.asarray(inputs["W1"], dtype=np.float32)             # [H, IN1]
    W2 = np.asarray(inputs["W2"], dtype=np.float32)             # [H, IN2]
    b2 = np.asarray(inputs["b2"], dtype=np.float32)             # [H]
    V = np.asarray(inputs["V"], dtype=np.float32)               # [H, 1]

    w1t = _to_bf16(W1.T)                                        # [IN1, H]
    w2ta = _to_bf16(np.concatenate([W2.T, b2[None, :]], 0))     # [IN2+1, H]
    vb = _to_bf16(np.broadcast_to(V.reshape(1, H), (128, H)))   # [128, H]

    in_maps = []
    for c in range(NCORES):
        enc_c = enc[c * BC:(c + 1) * BC]                        # [16, N, IN1]
        flat = enc_c.reshape(TOK, IN1)
        encT = _to_bf16(np.ascontiguousarray(flat.T))           # [IN1, TOK]
        encN = np.zeros((TOK, ENC_NW), dtype=ml_dtypes.bfloat16)
        encN[:, :IN1] = flat.astype(ml_dtypes.bfloat16)
        encN[:, IN1] = 1.0
        encN = encN.view(np.uint16)
        h0t = _to_bf16(h0[c * BC:(c + 1) * BC].T)               # [IN2, 16]
        in_maps.append({
            "encT": encT, "encN": encN, "w1t": w1t,
            "h0t": h0t, "w2ta": w2ta, "vb": vb,
        })

    if _CACHED_NC is None:
        _CACHED_NC = build_nc()
    nc = _CACHED_NC

    runner = Runner(nc, in_maps)
    LAST_RUNNER = runner
    results = runner.outputs(runner.run())
    out = np.concatenate([results[i]["out"] for i in range(NCORES)], axis=0)
    return out.astype(np.float32)


# revision 43
# speedup vs baseline: 1.0065x; 1.0000x over previous
"""Bass/Trainium2 kernel for nn_Attention (Bahdanau-style attention).

  w1e   = enc @ W1.T                      [B, N, H]
  w2h   = h0 @ W2.T + b2                  [B, H]
  u     = tanh(w1e + w2h[:, None, :])     [B, N, H]
  logits= u @ V                           [B, N, 1]
  att   = softmax(logits, axis=1)
  out   = att^T @ enc                     [B, IN1]

Sharding: pure data-parallel over batch B=128 across 8 cores (16 batches
each); W1/W2/V replicated. No collectives.

Per-core dataflow (layout: tokens on partitions, H on free dim):
  - main matmul: stationary = enc^T tile [128 IN1, 128 tok] (host
    pre-transposed, bf16 shipped as uint16), moving = W1^T [128 IN1,
    512 H]; K=IN1=256 -> 2 accumulating matmuls per token tile; a 3rd
    K=1 matmul with an all-ones stationary row folds c = W2 h0 + b2
    (computed on device, broadcast to 128 partitions via a DRAM bounce)
    into the same PSUM accumulation. The 4 c-fold matmuls of a 4-tile
    group are row-packed at tile_position rows {0,32,64,96} so they run
    concurrently on the PE array.
  - tanh on ScalarE (PSUM->SBUF, bf16 out), 2 token-tiles per instr.
  - V-dot: one pair-wide tensor_tensor mult (u * V via a stride-0
    middle-dim broadcast AP), mostly on GpSimd, then a free-dim
    add-reduce -> logits columns [128, 16]. Pure-DVE pairs use one 3D
    tensor_reduce ([128,2,512] -> [128,2]); ~24/256 reduces go to
    ScalarE Identity+accum_out so ScalarE and VectorE stay balanced.
    (tensor_tensor_reduce and tensor_scalar+accum_out are broken on
    this toolchain: device wedge / BIR verifier reject.)
  - exp on ScalarE -> e [128, 16] bf16 (no max-subtract: |logits| <=
    ||V||_1 ~= 18, exp fits fp32/bf16 fine).
  - final weighted sum on PE: stationary = e column [128, 1], moving =
    enc natural tile [128 tok, 257] where column 256 is all-ones ->
    psum [1, 257] accumulates both att^T@enc AND the softmax denominator.
  - normalize by 1/S on ScalarE, DMA out per batch row.
"""

import os
import sys

for _p in ("/opt/trn_rl_repo",):
    if _p not in sys.path and os.path.isdir(_p):
        sys.path.insert(0, _p)

from contextlib import ExitStack

import ml_dtypes
import numpy as np

import concourse.bass as bass
from concourse import bacc, mybir, tile

B, N, IN1, IN2, H = 128, 2048, 256, 512, 512
NCORES = 8
BC = B // NCORES            # 16 batches per core
TOK = BC * N                # 32768 tokens per core
TPB = N // 128              # 16 token tiles per batch
NPAIR = TPB // 2            # 8 tile-pairs per batch
ENC_NW = 272                # padded natural width (257 used, 32B-aligned rows)

F32 = mybir.dt.float32
BF16 = mybir.dt.bfloat16

LAST_RUNNER = None

_CACHED_NC = None


class Runner:
    """Compile-once SPMD runner (replicates run_bass_via_pjrt's multi-core
    path) that keeps the jitted callable + device-resident inputs so
    repeated executions can be wall-clocked without compile/transfer."""

    def __init__(self, nc, in_maps):
        import jax
        from jax.experimental.shard_map import shard_map
        from jax.sharding import Mesh, NamedSharding, PartitionSpec

        from concourse import bass2jax, mybir as _mybir

        bass2jax.install_neuronx_cc_hook()
        self.jax = jax

        if not nc.is_finalized():
            nc.finalize()

        partition_name = (nc.partition_id_tensor.name
                          if nc.partition_id_tensor else None)
        in_names, out_names, out_avals, zero_outs = [], [], [], []
        for alloc in nc.m.functions[0].allocations:
            if not isinstance(alloc, _mybir.MemoryLocationSet):
                continue
            name = alloc.memorylocations[0].name
            if alloc.kind == "ExternalInput":
                if name != partition_name:
                    in_names.append(name)
            elif alloc.kind == "ExternalOutput":
                shape = tuple(alloc.tensor_shape)
                dtype = _mybir.dt.np(alloc.dtype)
                out_names.append(name)
                out_avals.append(jax.core.ShapedArray(shape, dtype))
                zero_outs.append(np.zeros(shape, dtype))
        n_params = len(in_names)
        all_in_names = list(in_names) + list(out_names)
        if partition_name is not None:
            all_in_names.append(partition_name)
        self.out_names = out_names

        def _body(*args):
            operands = list(args)
            if partition_name is not None:
                operands.append(bass2jax.partition_id_tensor())
            outs = bass2jax._bass_exec_p.bind(
                *operands,
                out_avals=tuple(out_avals),
                in_names=tuple(all_in_names),
                out_names=tuple(out_names),
                lowering_input_output_aliases=(),
                sim_require_finite=True,
                sim_require_nnan=True,
                nc=nc,
            )
            return tuple(outs)

        n_cores = len(in_maps)
        devices = jax.devices()[:n_cores]
        mesh = Mesh(np.asarray(devices), ("core",))
        spec = PartitionSpec("core")
        self.n_cores = n_cores
        self.out_avals = out_avals
        self.sharded = jax.jit(
            shard_map(_body, mesh=mesh,
                      in_specs=(spec,) * (n_params + len(out_names)),
                      out_specs=(spec,) * len(out_names),
                      check_rep=False),
            keep_unused=True,
        )

        def _body_chain(k):
            # k sequential executions chained through the output buffers:
            # each call's outputs become the next call's pre-zeroed output
            # operands, forcing true sequential execution in one dispatch.
            def f(*args):
                ins, zouts = args[:n_params], list(args[n_params:])
                for _ in range(k):
                    zouts = list(_body(*ins, *zouts))
                return tuple(zouts)
            return f

        self._chain_cache = {}
        self._mesh, self._spec = mesh, spec
        self._n_params = n_params
        self._shard_map, self._jit = shard_map, jax.jit
        self._body_chain = _body_chain
        sharding = NamedSharding(mesh, spec)
        self.dev_in = [
            jax.device_put(
                np.concatenate([np.asarray(in_maps[c][nm])
                                for c in range(n_cores)], axis=0), sharding)
            for nm in in_names
        ]
        self.dev_zeros = [
            jax.device_put(
                np.zeros((n_cores * z.shape[0], *z.shape[1:]), z.dtype), sharding)
            for z in zero_outs
        ]

    def run(self):
        out = self.sharded(*self.dev_in, *self.dev_zeros)
        self.jax.block_until_ready(out)
        return out

    def run_chain(self, k):
        # k async dispatches of the same executable; PJRT serializes them
        # on the device stream, so wall(k) - wall(1) ~= (k-1) * exec_time
        # (neuronx_cc_hook rejects >1 bass_exec per jitted module, so a
        # true in-graph chain is not compilable).
        out = None
        for _ in range(k):
            out = self.sharded(*self.dev_in, *self.dev_zeros)
        self.jax.block_until_ready(out)
        return out

    def outputs(self, out_arrs):
        return [
            {nm: np.asarray(out_arrs[i]).reshape(
                self.n_cores, *self.out_avals[i].shape)[c]
             for i, nm in enumerate(self.out_names)}
            for c in range(self.n_cores)
        ]


def build_nc(bc=BC, tpb=TPB):
    tok = bc * tpb * 128
    npair = tpb // 2
    nc = bacc.Bacc(None, target_bir_lowering=False)

    # NOTE: native bfloat16 ExternalInputs are mangled by the axon/PJRT
    # transfer path (measured: garbage values, device wedge). Ship bf16
    # bits as uint16 and bitcast on-chip.
    U16 = mybir.dt.uint16
    encT = nc.dram_tensor("encT", [IN1, tok], U16, kind="ExternalInput")
    encN = nc.dram_tensor("encN", [tok, ENC_NW], U16, kind="ExternalInput")
    w1t = nc.dram_tensor("w1t", [IN1, H], U16, kind="ExternalInput")
    h0t = nc.dram_tensor("h0t", [IN2, bc], U16, kind="ExternalInput")
    w2ta = nc.dram_tensor("w2ta", [IN2 + 1, H], U16, kind="ExternalInput")
    vb = nc.dram_tensor("vb", [128, H], U16, kind="ExternalInput")
    out = nc.dram_tensor("out", [bc, IN1], F32, kind="ExternalOutput")

    Tanh = mybir.ActivationFunctionType.Tanh
    Exp = mybir.ActivationFunctionType.Exp
    Copy = mybir.ActivationFunctionType.Copy
    Alu = mybir.AluOpType

    with tile.TileContext(nc) as tc, ExitStack() as ctx:
        consts = ctx.enter_context(tc.tile_pool(name="consts", bufs=1))
        etp = ctx.enter_context(tc.tile_pool(name="etp", bufs=3))
        enp = ctx.enter_context(tc.tile_pool(name="enp", bufs=6))
        upool = ctx.enter_context(tc.tile_pool(name="upool", bufs=4))
        lpool = ctx.enter_context(tc.tile_pool(name="lpool", bufs=2))
        epool = ctx.enter_context(tc.tile_pool(name="epool", bufs=2))
        spool = ctx.enter_context(tc.tile_pool(name="spool", bufs=4))
        zpool = ctx.enter_context(tc.tile_pool(name="zpool", bufs=3, space="PSUM"))
        opool = ctx.enter_context(tc.tile_pool(name="opool", bufs=2, space="PSUM"))

        # ---------------- prologue: constants ----------------
        sb_w1t = consts.tile([128, 2, H], BF16)
        for k in range(2):
            nc.sync.dma_start(out=sb_w1t[:, k, :].bitcast(U16),
                              in_=w1t[k * 128:(k + 1) * 128, :])
        sb_vb = consts.tile([128, H], BF16)
        sb_ones = consts.tile([1, 128], BF16)
        nc.vector.memset(sb_ones, 1.0)

        sb_h0t = consts.tile([128, 4, bc], BF16)
        for k in range(4):
            nc.sync.dma_start(out=sb_h0t[:, k, :].bitcast(U16),
                              in_=h0t[k * 128:(k + 1) * 128, :])
        sb_w2ta = consts.tile([128, 5, H], BF16)
        for k in range(4):
            nc.sync.dma_start(out=sb_w2ta[:, k, :].bitcast(U16),
                              in_=w2ta[k * 128:(k + 1) * 128, :])
        nc.sync.dma_start(out=sb_w2ta[0:1, 4, :].bitcast(U16),
                          in_=w2ta[IN2:IN2 + 1, :])

        # c = h0 @ W2.T + b2  -> [16, 512] in PSUM
        psum_c = zpool.tile([bc, H], F32, tag="z")
        for k in range(4):
            nc.tensor.matmul(psum_c, sb_h0t[:, k, :], sb_w2ta[:, k, :],
                             start=(k == 0), stop=False)
        nc.tensor.matmul(psum_c, sb_ones[0:1, 0:bc], sb_w2ta[0:1, 4, :],
                         start=False, stop=True)
        sb_c16 = consts.tile([bc, H], BF16)
        nc.vector.tensor_copy(sb_c16, psum_c)
        # c rows -> DRAM bounce -> broadcast to all 128 partitions, so the
        # K=1 c-fold matmuls can be row-packed at tile_position rows
        # {0,32,64,96} (4 concurrent on the PE array).
        dpool = ctx.enter_context(tc.tile_pool(name="dpool", bufs=1, space="DRAM"))
        c_dram = dpool.tile([bc, H], BF16)
        nc.gpsimd.dma_start(out=c_dram[:, :], in_=sb_c16[:, :])
        crep = consts.tile([128, bc * H], BF16)
        c_flat0 = bass.AP(tensor=c_dram.tensor, offset=c_dram.offset,
                          ap=[[0, 128], [1, H]])
        nc.gpsimd.dma_start(out=crep[:, 0:H], in_=c_flat0)
        c_flat1 = bass.AP(tensor=c_dram.tensor, offset=c_dram.offset + H,
                          ap=[[0, 128], [1, (bc - 1) * H]])
        nc.gpsimd.dma_start(out=crep[:, H:], in_=c_flat1)
        sb_onesq = consts.tile([128, 128], BF16)
        nc.vector.memset(sb_onesq, 1.0)
        nc.sync.dma_start(out=sb_vb.bitcast(U16), in_=vb[:, :])

        # ---------------- main pipeline ----------------
        for b in range(bc):
            sb_logits = lpool.tile([128, tpb], F32, tag="logits")
            for g8 in range(tpb // 8):             # 8 token tiles per DMA group
                tok8 = (b * tpb + g8 * 8) * 128
                sb_et = etp.tile([128, 2, 1024], BF16, tag="et")
                for k in range(2):
                    nc.sync.dma_start(
                        out=sb_et[:, k, :].bitcast(U16),
                        in_=encT[k * 128:(k + 1) * 128, tok8:tok8 + 1024])
                for sub in range(2):                # 4-tile compute sub-groups
                    grp = g8 * 2 + sub
                    pz0 = zpool.tile([128, 1024], F32, tag="z")
                    pz1 = zpool.tile([128, 1024], F32, tag="z")
                    pz = [pz0, pz1]
                    for q in range(4):              # main matmuls, 4 tiles
                        zs = pz[q // 2][:, (q % 2) * 512:(q % 2 + 1) * 512]
                        qq = sub * 4 + q
                        for k in range(2):
                            nc.tensor.matmul(
                                zs, sb_et[:, k, qq * 128:(qq + 1) * 128],
                                sb_w1t[:, k, :], start=(k == 0), stop=False)
                    first_grp = (b == 0 and g8 == 0 and sub == 0)
                    for q in range(4):              # row-packed c-fold matmuls
                        zs = pz[q // 2][:, (q % 2) * 512:(q % 2 + 1) * 512]
                        if first_grp:
                            # first group reads c directly from sb_c16 row 0
                            # (partition 0), skipping the crep DMA-chain
                            # latency at kernel startup
                            nc.tensor.matmul(zs, sb_onesq[0:1, :],
                                             sb_c16[0:1, :],
                                             start=False, stop=True)
                        else:
                            nc.tensor.matmul(zs, sb_onesq[32 * q:32 * q + 1, :],
                                             crep[32 * q:32 * q + 1,
                                                  b * H:(b + 1) * H],
                                             start=False, stop=True,
                                             tile_position=(32 * q, 0))
                    for hz in range(2):
                        sb_u = upool.tile([128, 1024], BF16, tag="u")
                        nc.scalar.activation(sb_u, pz[hz], Tanh)
                        sb_prod = upool.tile([128, 2, 512], BF16, tag="prod")
                        t0 = grp * 4 + hz * 2
                        # one pair-wide V multiply (V_bcast repeated along free)
                        # last batch's multiplies on VectorE: the tail
                        # chain (tanh->mult->reduce->exp->finals) is serial,
                        # and gpsimd adds queue-hop latency there
                        mul_eng = (nc.vector if (b == bc - 1 and g8 == 1)
                                   else nc.gpsimd if (t0 // 2) % 4 != 3
                                   else nc.vector)
                        mul_eng.tensor_tensor(
                            out=sb_prod,
                            in0=sb_u.rearrange("p (j f) -> p j f", j=2),
                            in1=bass.AP(tensor=sb_vb.tensor,
                                        offset=sb_vb.offset,
                                        ap=[sb_vb.ap[0], [0, 2],
                                            sb_vb.ap[1]]),
                            op=Alu.mult)
                        # ScalarE takes tile 5 every batch and tile 11 on
                        # odd batches (~24/256 reduces) to balance ACT~DVE
                        act_tiles = {5} if b % 2 == 0 else {5, 11}
                        if (t0 in act_tiles) or (t0 + 1 in act_tiles):
                            # mixed pair: per-tile reduces (one on ScalarE)
                            for half in range(2):
                                t_idx = t0 + half
                                lg = sb_logits[:, t_idx:t_idx + 1]
                                if t_idx in act_tiles:
                                    junk = upool.tile([128, 512], BF16,
                                                      tag="junk")
                                    nc.scalar.activation(
                                        junk, sb_prod[:, half, :],
                                        mybir.ActivationFunctionType.Identity,
                                        accum_out=lg)
                                else:
                                    nc.vector.tensor_reduce(
                                        out=lg, in_=sb_prod[:, half, :],
                                        op=Alu.add, axis=mybir.AxisListType.X)
                        else:
                            # pure-DVE pair: one 3D reduce -> two logit cols
                            nc.vector.tensor_reduce(
                                out=sb_logits[:, t0:t0 + 2], in_=sb_prod,
                                op=Alu.add, axis=mybir.AxisListType.X)

            sb_e = epool.tile([128, tpb], BF16, tag="e")
            if b == bc - 1:
                # last batch: exp in two chunks so its final matmuls start
                # before the last logits are reduced (shortens the drain tail)
                half_t = tpb // 2
                nc.scalar.activation(sb_e[:, 0:half_t],
                                     sb_logits[:, 0:half_t], Exp)
                nc.scalar.activation(sb_e[:, half_t:],
                                     sb_logits[:, half_t:], Exp)
            else:
                nc.scalar.activation(sb_e, sb_logits, Exp)

            psum_o = opool.tile([1, 257], F32, tag="o")
            for sg in range(tpb // 4):
                s0 = b * tpb + sg * 4
                sb_en = enp.tile([128, 4, ENC_NW], BF16, tag="en")
                nc.sync.dma_start(
                    out=sb_en.bitcast(U16),
                    in_=encN[s0 * 128:(s0 + 4) * 128, :].rearrange(
                        "(j p) c -> p j c", p=128))
                for j in range(4):
                    s = sg * 4 + j
                    nc.tensor.matmul(psum_o, sb_e[:, s:s + 1],
                                     sb_en[:, j, 0:257],
                                     start=(s == 0), stop=(s == tpb - 1))
            rS = spool.tile([1, 1], F32, tag="rs")
            nc.vector.reciprocal(rS, psum_o[0:1, 256:257])
            sb_out = spool.tile([1, IN1], F32, tag="obuf")
            nc.vector.tensor_scalar_mul(sb_out, psum_o[0:1, 0:256], rS)
            nc.sync.dma_start(out=out[b:b + 1, :], in_=sb_out)

    return nc


def _to_bf16(x):
    """bf16 bits as uint16 (native bf16 inputs are mangled by the
    transfer path - see build_nc note)."""
    return np.ascontiguousarray(x.astype(ml_dtypes.bfloat16)).view(np.uint16)


def kernel(**inputs):
    global LAST_RUNNER, _CACHED_NC
    enc = np.asarray(inputs["enc_outputs"], dtype=np.float32)   # [B, N, IN1]
    h0 = np.asarray(inputs["h0"], dtype=np.float32)             # [B, IN2]
    W1 = np.asarray(inputs["W1"], dtype=np.float32)             # [H, IN1]
    W2 = np.asarray(inputs["W2"], dtype=np.float32)             # [H, IN2]
    b2 = np.asarray(inputs["b2"], dtype=np.float32)             # [H]
    V = np.asarray(inputs["V"], dtype=np.float32)               # [H, 1]

    w1t = _to_bf16(W1.T)                                        # [IN1, H]
    w2ta = _to_bf16(np.concatenate([W2.T, b2[None, :]], 0))     # [IN2+1, H]
    vb = _to_bf16(np.broadcast_to(V.reshape(1, H), (128, H)))   # [128, H]

    in_maps = []
    for c in range(NCORES):
        enc_c = enc[c * BC:(c + 1) * BC]                        # [16, N, IN1]
        flat = enc_c.reshape(TOK, IN1)
        encT = _to_bf16(np.ascontiguousarray(flat.T))           # [IN1, TOK]
        encN = np.zeros((TOK, ENC_NW), dtype=ml_dtypes.bfloat16)
        encN[:, :IN1] = flat.astype(ml_dtypes.bfloat16)
        encN[:, IN1] = 1.0
        encN = encN.view(np.uint16)
        h0t = _to_bf16(h0[c * BC:(c + 1) * BC].T)               # [IN2, 16]
        in_maps.append({
            "encT": encT, "encN": encN, "w1t": w1t,
            "h0t": h0t, "w2ta": w2ta, "vb": vb,
        })

    if _CACHED_NC is None:
        _CACHED_NC = build_nc()
    nc = _CACHED_NC

    runner = Runner(nc, in_maps)
    LAST_RUNNER = runner
    results = runner.outputs(runner.run())
    out = np.concatenate([results[i]["out"] for i in range(NCORES)], axis=0)
    return out.astype(np.float32)
